# revision 1
# baseline (speedup 1.0000x reference)
"""GAT (graph attention) layer on 8 Trainium2 NeuronCores.

Reference computation (N=8192, F_IN=256, F_OUT=64, alpha=0.2):
    Wh     = h @ W                                  [N, 64]
    f_src  = Wh @ a[:64, 0]                         [N]
    f_dst  = Wh @ a[64:, 0]                         [N]
    e      = leaky_relu(f_src[:,None] + f_dst[None,:], 0.2)
    att    = softmax(where(adj > 0, e, -9e15), axis=1)
    out    = elu(att @ Wh)

Sharding strategy (host-side input marshalling inside kernel()):
  Row-shard the N query dimension across 8 cores (1024 rows/core). Each
  core receives its adjacency shard TRANSPOSED and cast to bf16
  (adjT[j, i] = adj[i, j]; exact for 0/1 values, halves DMA bytes) plus
  the full h transposed in bf16 (hT), so the device program needs NO
  N^2 PE transposes and no PSUM->SBUF staging copies for the attention
  matrix; the adjacency stream is the memory roofline (~47us/core).

Device algebra:
  exp(lrelu(u)) with u = fs_i + fd_j factors as
     exp(0.2 fs_i) * max(u_i v_j, c_j),
     c_j = exp(0.2 fd_j), u_i = exp(0.8 fs_i), v_j = exp(fd_j).
  The row-constant exp(0.2 fs_i) cancels in the softmax. The c_j factor
  rides inside the clamp (second scalar slot of tensor_scalar, a
  per-partition AP), so the matmul rhs is just [Wh_j | 1] bf16 - one
  batched plain PSUM->SBUF copy per 4-chunk group and a memset ones-
  column that also produces the softmax denominator Z.

  Main loop per j-chunk (layout [j=128 partitions, i=1024 free], bf16):
    Xm = max(u_bcast * v_j, c_j)    1 DVE tensor_scalar (4x fast mode)
    p  = Xm * adjT                  1 tensor_tensor (DVE/Pool interleave)
    accT[f, i] += rhs_aug[j, f]^T @ p[j, i]   2 bf16 matmuls (1 cyc/row)
  The masked-score matrix is produced directly in the [j, i] layout the
  contraction needs, with the clamp fused into the rank-1 product, so
  the N^2 elementwise pipeline is 2 ops/element (vs 5 in a naive port),
  split across DVE and Pool to keep both below the DMA rate.

  Overlap/queues: adjT streams on the SP hwdge queue in 2-chunk (512KB)
  DMAs (singles for the final group to shorten the tail); hT/W/a/out
  issue from the Activation hwdge queue so the two streams never
  head-of-line block each other. Wh -> rhs_aug/v08 production is
  interleaved with the attention loop per 4-chunk group. Epilogue:
  one [65,512] PSUM->SBUF copy per accumulator half (ACT for one, DVE
  for the other), 4 PE transposes each, then batched [128,4,64] vector
  ops; ELU as max(exp(min(x,0)) - 1, x).

  Engine notes baked in: TensorScalarPtr ops are DVE-only on HW (Pool
  rejects them; Pool only runs plain tensor_tensor here) and GPSIMD
  cannot touch PSUM. ACT stays on the exp/copy activation table the
  whole kernel (no table-swap stalls).
"""

import sys

sys.path.insert(0, "/opt/trn_rl_repo")

import os

import numpy as np
import ml_dtypes

import concourse.bass as bass  # noqa: F401
import concourse.mybir as mybir
import concourse.tile as tile
from concourse import bacc
from concourse.bass_utils import run_bass_kernel_spmd
from concourse.masks import make_identity

N = 8192
F_IN = 256
F_OUT = 64
N_CORES = 8
ROWS = N // N_CORES  # 1024 query rows per core
MCH = N // 128  # 64 j-chunks
LCH = ROWS // 128  # 8 i-blocks
KCH = F_IN // 128  # 2 contraction chunks

F32 = mybir.dt.float32
BF16 = mybir.dt.bfloat16
I32 = mybir.dt.int32
Act = mybir.ActivationFunctionType
Alu = mybir.AluOpType
NPBF16 = ml_dtypes.bfloat16

_CACHE = {}


def _build_nc():
    nc = bacc.Bacc(
        "TRN2",
        target_bir_lowering=False,
        debug=False,
        enable_asserts=False,
        num_devices=N_CORES,
    )

    hT = nc.dram_tensor("hT", [F_IN, N], BF16, kind="ExternalInput")
    hsT = nc.dram_tensor("hsT", [F_IN, ROWS], BF16, kind="ExternalInput")
    adjT = nc.dram_tensor("adjT", [N, ROWS], BF16, kind="ExternalInput")
    W = nc.dram_tensor("W", [F_IN, F_OUT], F32, kind="ExternalInput")
    a = nc.dram_tensor("a", [2 * F_OUT, 1], F32, kind="ExternalInput")
    out = nc.dram_tensor("out", [ROWS, F_OUT], F32, kind="ExternalOutput")
    fsd = nc.dram_tensor("fsd", [ROWS], BF16)  # u = exp(.8 fs) bounce

    with tile.TileContext(nc) as tc:
        _kernel_body(nc, tc, hT, hsT, adjT, W, a, out, fsd)

    nc.compile()
    return nc


def _kernel_body(nc, tc, hT, hsT, adjT, W, a, out, fsd):
    # ACT-path fraction: chunks with (mc % 8) < ACT_NUM use the ACT Exp
    # path; the rest compute the rank-1 product on DVE.
    ACT_NUM = int(os.environ.get("GAT_ACT_NUM", "6"))
    ADJBUFS = int(os.environ.get("GAT_ADJBUFS", "5"))  # 4-chunk tiles
    WKBUFS = int(os.environ.get("GAT_WKBUFS", "6"))
    GRP = 4  # Wh chunks per pre-phase group
    EPBUFS = int(os.environ.get("GAT_EPBUFS", "4"))

    with (
        tc.tile_pool(name="consts", bufs=1) as consts,
        tc.tile_pool(name="adjp", bufs=ADJBUFS) as adjp,
        tc.tile_pool(name="wk", bufs=WKBUFS) as wk,
        tc.tile_pool(name="ep", bufs=EPBUFS) as ep,
        tc.tile_pool(name="psS", bufs=2, space="PSUM") as psS,
        tc.tile_pool(name="psS2", bufs=2, space="PSUM") as psS2,
        tc.tile_pool(name="psW", bufs=2, space="PSUM") as psW,
        tc.tile_pool(name="psACC", bufs=1, space="PSUM") as psACC,
    ):
        # ---------------- constants ----------------
        idf = consts.tile([128, 128], F32)
        make_identity(nc, idf)

        # Waug = [W | w_src | w_dst] as [128, 2, 66] f32, then bf16 copy
        Waug = consts.tile([128, 2, 66], F32)
        nc.scalar.dma_start(
            out=Waug[:, :, 0:F_OUT],
            in_=W[:, :].rearrange("(c p) f -> p c f", p=128),
        )
        a2 = consts.tile([64, 2], F32)
        nc.scalar.dma_start(out=a2, in_=a.rearrange("(c p) x -> p (c x)", p=F_OUT))

        WTs = consts.tile([64, 2, 128], F32)
        for rc in range(KCH):
            scr = psS.tile([128, 128], F32, tag="s")
            wtps = scr[0:64, :]
            nc.tensor.transpose(wtps, Waug[:, rc, 0:F_OUT], idf)
            nc.vector.tensor_copy(WTs[:, rc, :], wtps)
        for rc in range(KCH):
            scr2 = psS.tile([128, 128], F32, tag="s")
            wps = scr2[:, 0:2]
            nc.tensor.matmul(wps, lhsT=WTs[:, rc, :], rhs=a2, start=True, stop=True)
            nc.vector.tensor_copy(Waug[:, rc, F_OUT : F_OUT + 2], wps)
        Waug_bf = consts.tile([128, 2, 66], BF16)
        nc.vector.tensor_copy(Waug_bf, Waug)

        # ---------------- own-row f_src -> broadcast row ----------------
        hsTs = consts.tile([128, 2, ROWS], BF16)
        for kc in range(KCH):
            nc.scalar.dma_start(
                out=hsTs[:, kc, :], in_=hsT[kc * 128 : (kc + 1) * 128, :]
            )
        scr3 = psS.tile([128, 128], F32, tag="s")
        fso_ps = scr3[:, 0:LCH]
        for lc in range(LCH):
            for kc in range(KCH):
                nc.tensor.matmul(
                    fso_ps[:, lc : lc + 1],
                    lhsT=hsTs[:, kc, lc * 128 : (lc + 1) * 128],
                    rhs=Waug_bf[:, kc, F_OUT : F_OUT + 1],
                    start=(kc == 0),
                    stop=(kc == KCH - 1),
                )
        # u_col = exp(0.8 fs) per own row, bounced through DRAM (written
        # with a transposed access pattern) and read back as a
        # partition-broadcast free-axis row u_b [128, 1024] (bf16).
        uo = consts.tile([128, LCH], BF16)
        nc.scalar.activation(uo, fso_ps, Act.Exp, bias=0.0, scale=0.8)
        nc.gpsimd.dma_start(out=fsd[:].rearrange("(q p) -> p q", p=128), in_=uo)
        u_b = consts.tile([128, ROWS], BF16)
        fsd_bc = bass.AP(tensor=fsd, offset=0, ap=[[0, 128], [1, ROWS]])
        nc.gpsimd.dma_start(out=u_b, in_=fsd_bc)

        # ---------------- Wh -> rhs_aug, interleaved with attention ------
        # DMA batching: HWDGE costs ~625ns per DMA issue (shared), so adjT
        # comes in 16 DMAs of 4 j-chunks and hT in 8 DMAs of 2048 columns.
        hTs = consts.tile([128, 2, N], BF16)
        bmat = consts.tile([128, MCH], F32)
        v08 = consts.tile([128, MCH], F32)
        rhs_aug = consts.tile([128, MCH, 65], BF16)
        nc.vector.memset(rhs_aug[:, :, F_OUT], 1.0)
        # mask-mult engine split: NPOOL of every 8 chunks go to Pool
        NPOOL = int(os.environ.get("GAT_NPOOL", "5"))
        slots_env = os.environ.get("GAT_SLOTS", "1,2,4,5,7")
        if slots_env:
            pool_slots = {int(x) for x in slots_env.split(",")}
        else:
            pool_slots = (
                {int((i + 0.5) * 8 / NPOOL) % 8 for i in range(NPOOL)}
                if NPOOL
                else set()
            )
        POOL16 = int(os.environ.get("GAT_POOL16", "-1"))  # extra pool slot per 16
        ACTDMA = int(os.environ.get("GAT_ACTDMA", "3"))  # adjT DMAs moved to ACT queue
        RHSDVE = int(os.environ.get("GAT_RHSDVE", "-1"))  # rhs copy index on DVE
        SUBSZ = int(os.environ.get("GAT_SUBSZ", "2"))  # chunks per adjT DMA

        accL = psACC.tile([65, 512], F32, tag="accL")
        accR = psACC.tile([65, 512], F32, tag="accR")

        for s in range(4):  # 4 slices of 16 j-chunks each
            for kc in range(KCH):
                nc.scalar.dma_start(
                    out=hTs[:, kc, s * 2048 : (s + 1) * 2048],
                    in_=hT[kc * 128 : (kc + 1) * 128, s * 2048 : (s + 1) * 2048],
                )
            for g in range(4):  # 4 groups of GRP=4 chunks; adjT in 2-chunk DMAs
                mc0 = s * 16 + g * GRP
                adjt = adjp.tile([128, GRP, ROWS], BF16, tag="adj")
                sub = 1 if mc0 == MCH - GRP else SUBSZ
                gidx = s * 4 + g
                for h2 in range(GRP // sub):
                    m2 = mc0 + h2 * sub
                    # optionally spread part of the adjacency stream onto the
                    # ACT hwdge queue to balance the two DMA rings
                    q_act = (
                        ACTDMA > 0
                        and h2 == min(1, GRP // sub - 1)
                        and gidx % (16 // ACTDMA) == 16 // ACTDMA - 2
                    )
                    dq = nc.scalar if q_act else nc.sync
                    dq.dma_start(
                        out=adjt[:, h2 * sub : (h2 + 1) * sub, :],
                        in_=adjT[m2 * 128 : (m2 + sub) * 128, :].rearrange(
                            "(c p) i -> p c i", p=128
                        ),
                    )
                # Wh / rhs_aug for these 4 chunks
                whps = psW.tile([128, GRP, 66], F32, tag="wh")
                for q in range(GRP):
                    mc = mc0 + q
                    for kc in range(KCH):
                        nc.tensor.matmul(
                            whps[:, q, :],
                            lhsT=hTs[:, kc, mc * 128 : (mc + 1) * 128],
                            rhs=Waug_bf[:, kc, :],
                            start=(kc == 0),
                            stop=(kc == KCH - 1),
                        )
                sl = slice(mc0, mc0 + GRP)
                nc.scalar.activation(
                    bmat[:, sl], whps[:, :, 65], Act.Exp, bias=0.0, scale=0.2
                )
                nc.scalar.activation(
                    v08[:, sl], whps[:, :, 65], Act.Exp, bias=0.0, scale=1.0
                )
                nc.scalar.activation(
                    rhs_aug[:, sl, 0:F_OUT], whps[:, :, 0:F_OUT], Act.Copy
                )
                # attention for these 4 chunks
                for q in range(GRP):
                    mc = mc0 + q
                    Xm = wk.tile([128, ROWS], BF16, tag="X")
                    nc.vector.tensor_scalar(
                        Xm,
                        u_b,
                        v08[:, mc : mc + 1],
                        bmat[:, mc : mc + 1],
                        Alu.mult,
                        Alu.max,
                    )
                    p = wk.tile([128, ROWS], BF16, tag="p")
                    if mc % 8 in pool_slots or mc % 16 == POOL16:
                        nc.gpsimd.tensor_tensor(p, Xm, adjt[:, q, :], Alu.mult)
                    else:
                        nc.vector.tensor_tensor(p, Xm, adjt[:, q, :], Alu.mult)
                    nc.tensor.matmul(
                        accL,
                        lhsT=rhs_aug[:, mc, :],
                        rhs=p[:, 0:512],
                        start=(mc == 0),
                        stop=(mc == MCH - 1),
                    )
                    nc.tensor.matmul(
                        accR,
                        lhsT=rhs_aug[:, mc, :],
                        rhs=p[:, 512:1024],
                        start=(mc == 0),
                        stop=(mc == MCH - 1),
                    )

        # ---------------- epilogue: transpose back + softmax-div + ELU ----
        # two quads of 128-row blocks: one [65, 512] PSUM->SBUF copy, four
        # transposes, then batched [128, 4, 64] vector ops.
        for qd in range(2):
            acc = accL if qd == 0 else accR
            sT4 = ep.tile([65, 512], F32, tag="sT")
            if qd == 0:
                nc.scalar.activation(sT4, acc, Act.Copy)
            else:
                nc.vector.tensor_copy(sT4, acc)
            scr6 = psS2.tile([128, 4, 65], F32, tag="s2")
            for j in range(4):
                nc.tensor.transpose(
                    scr6[:, j, :], sT4[:, j * 128 : (j + 1) * 128], idf[0:65, 0:65]
                )
            rz4 = ep.tile([128, 4], F32, tag="rz")
            nc.vector.reciprocal(rz4, scr6[:, :, F_OUT])
            sc4 = ep.tile([128, 4, F_OUT], F32, tag="sc")
            rz4bc = rz4.unsqueeze(-1).broadcast_to([128, 4, F_OUT])
            nc.vector.tensor_tensor(sc4, scr6[:, :, 0:F_OUT], rz4bc, Alu.mult)
            mn4 = ep.tile([128, 4, F_OUT], F32, tag="mn")
            nc.vector.tensor_scalar(mn4, sc4, 0.0, None, Alu.min)
            em4 = ep.tile([128, 4, F_OUT], F32, tag="em")
            nc.scalar.activation(em4, mn4, Act.Exp, bias=0.0, scale=1.0)
            # elu(x) = max(exp(min(x,0)) - 1, x)
            ob4 = ep.tile([128, 4, F_OUT], F32, tag="ob")
            nc.vector.scalar_tensor_tensor(ob4, em4, -1.0, sc4, Alu.add, Alu.max)
            oq = nc.scalar if qd == 0 else nc.sync
            oq.dma_start(
                out=out[qd * 512 : (qd + 1) * 512, :].rearrange(
                    "(c p) f -> p c f", p=128
                ),
                in_=ob4,
            )


def _get_nc():
    key = (
        "nc2",
        os.environ.get("GAT_NPOOL", ""),
        os.environ.get("GAT_ADJBUFS", ""),
        os.environ.get("GAT_WKBUFS", ""),
        os.environ.get("GAT_ACTDMA", ""),
        os.environ.get("GAT_SLOTS", ""),
        os.environ.get("GAT_POOL16", ""),
    )
    if key not in _CACHE:
        _CACHE[key] = _build_nc()
    return _CACHE[key]


def make_in_maps(h, adj, W, a):
    h = np.ascontiguousarray(h, dtype=np.float32)
    W = np.ascontiguousarray(W, dtype=np.float32)
    a = np.ascontiguousarray(a, dtype=np.float32)

    hT = np.ascontiguousarray(h.T.astype(NPBF16))
    in_maps = []
    for c in range(N_CORES):
        sl = slice(c * ROWS, (c + 1) * ROWS)
        in_maps.append(
            {
                "hT": hT,
                "hsT": np.ascontiguousarray(h[sl].T.astype(NPBF16)),
                "adjT": adj[sl].T.astype(NPBF16),
                "W": W,
                "a": a,
            }
        )
    return in_maps


def kernel(h, adj, W, a, _collect_results=False, _trace=False):
    nc = _get_nc()
    in_maps = make_in_maps(h, adj, W, a)
    res = run_bass_kernel_spmd(nc, in_maps, list(range(N_CORES)), trace=_trace)
    out = np.concatenate([res.results[c]["out"] for c in range(N_CORES)], axis=0)
    out = np.ascontiguousarray(out, dtype=np.float32)
    if _collect_results:
        return out, res
    return out

